# revision 28
# baseline (speedup 1.0000x reference)
"""DualTransformerBlock Trainium2 kernel (v2 — dual-stream, AllGather).

Distribution: 2 replica groups of 4 cores. Group g owns samples {2g, 2g+1};
core q within the group owns token quarter q (1024 tokens) of BOTH samples.
Each core runs two independent dependency chains ("streams" A/B, one per
sample); the Tile scheduler interleaves them so one stream's collectives
hide under the other stream's compute.

Key optimizations over v1:
  - AllGather (no 1.875x AllReduce multiplier in HW) + local sum instead of
    AllReduce for the tiny cross-core reductions (EA context matrix,
    channel-attn gram/norms).
  - fp16 activations/weights everywhere (PSUM stays f32).
  - LayerNorm: bn_stats + fast inverse-sqrt on DVE (no Act sqrt tables) +
    one fused scale/bias tensor_scalar per tile; token->channel-major
    transposes done by the DMA transpose crossbar (frees PE/DVE/Act).
  - EfficientAttention: att = n1 @ (wv_g @ S2) fold — V is never
    materialized.  ChannelAttention: out = (attn^T P) applied to v_cm fold —
    separate attn@v and proj matmuls are merged.
  - Act engine only ever runs Exp and Gelu (plus table-free Identity), so
    at most ~2 activation-table loads.
"""

import sys

sys.path.insert(0, "/opt/trn_rl_repo")

import numpy as np

import concourse.bass as bass
import concourse.mybir as mybir
from concourse import bacc
from concourse.tile import TileContext

F32 = mybir.dt.float32
F16 = mybir.dt.float16
F8 = mybir.dt.float8e4
I32 = mybir.dt.int32
AF = mybir.ActivationFunctionType
OP = mybir.AluOpType
AX = mybir.AxisListType

B, N, C = 4, 4096, 256
H_CH = 8
HD = C // H_CH          # 32
DFF = 4 * C             # 1024
EPS_LN = 1e-5

NCORES = 8
DUAL = False            # True: 2 streams/core, 4-core groups; False: 1 stream, pairs
STREAMS = "ab" if DUAL else "a"
NRANKS = 4 if DUAL else 2
TQ = N // NRANKS        # tokens per stream per core
NT = TQ // 128          # token tiles
CT = C // 128           # 2 channel tiles
FT = DFF // 128         # 8 ff tiles
NCH = TQ // 512         # free-dim chunks of 512
REPLICA_GROUPS = ([[0, 1, 2, 3], [4, 5, 6, 7]] if DUAL else
                  [[0, 1], [2, 3], [4, 5], [6, 7]])
RSQRT_MAGIC = 0x5F3759DF

_CACHE = {}


def build_program():
    if "nc" in _CACHE:
        return _CACHE["nc"]
    nc = bacc.Bacc(None, target_bir_lowering=False)

    io = {}

    def param(name, shape, dt=F16):
        io[name] = nc.declare_dram_parameter(name, list(shape), dt, isOutput=False)

    for s in STREAMS:
        param(f"x_{s}", (TQ, C))
    for nm, shape in [
        ("wkq_t", (C, 2 * C)), ("wr_t", (C, C)), ("wv_t", (C, C)),
        ("qk_t", (C, 2 * C)), ("v_t", (C, C)), ("p_t", (C, C)),
        ("w1_t", (C, DFF)),
        ("w3_t", (C, DFF)),
        ("ident", (128, 128)), ("ones_pc", (128, 1)), ("ones_pr", (1, 128)),
    ]:
        param(nm, shape)
    param("w2_t", (DFF, C), F8)
    param("w4_t", (DFF, C), F8)
    param("temp_c", (128, CT), F32)
    for s in STREAMS:
        io[f"y_{s}"] = nc.declare_dram_parameter(f"y_{s}", [TQ, C], F32, isOutput=True)

    cc = {}
    for s in STREAMS:
        cc[f"ea_in_{s}"] = nc.dram_tensor(f"ea_in_{s}", [128 * 2 * C], F8)
        cc[f"ea_out_{s}"] = nc.dram_tensor(
            f"ea_out_{s}", [NRANKS * 128 * 2 * C], F8)
        W_CA = 2 * HD + 2 * CT
        cc[f"ca_in_{s}"] = nc.dram_tensor(f"ca_in_{s}", [128 * W_CA], F16)
        cc[f"ca_out_{s}"] = nc.dram_tensor(
            f"ca_out_{s}", [NRANKS * 128 * W_CA], F16)

    with TileContext(nc) as tc:
        with (
            tc.tile_pool(name="wpool", bufs=1) as wp,
            tc.tile_pool(name="apool", bufs=1) as ap,
            tc.tile_pool(name="tmp", bufs=3) as tp,
            tc.tile_pool(name="stage", bufs=1) as stg,
            tc.tile_pool(name="pacc", bufs=1, space="PSUM") as pacc,
            tc.tile_pool(name="pmm", bufs=5, space="PSUM") as pmm,
        ):
            # ---------------- inputs + consts ----------------
            x_sb = {}
            for s in STREAMS:
                x_sb[s] = ap.tile([128, NT, C], F16, tag=f"resid_{s}", bufs=2,
                                  name=f"x_sb_{s}")
                xr = io[f"x_{s}"][:, :].rearrange("(p t) c -> p t c", p=128)
                qn = NT // 4
                for qq in range(4):
                    nc.sync.dma_start(out=x_sb[s][:, qq * qn:(qq + 1) * qn, :],
                                      in_=xr[:, qq * qn:(qq + 1) * qn, :])

            ident = wp.tile([128, 128], F16, tag="ident")
            nc.gpsimd.dma_start(out=ident, in_=io["ident"][:, :])
            ident32 = wp.tile([128, 128], F32, tag="ident32")
            nc.scalar.activation(ident32, ident, AF.Identity)
            ones_col = wp.tile([128, 1], F16, tag="ones_col")
            nc.gpsimd.dma_start(out=ones_col, in_=io["ones_pc"][:, :])
            ones_row = wp.tile([1, 128], F16, tag="ones_row")
            nc.gpsimd.dma_start(out=ones_row, in_=io["ones_pr"][:, :])
            temp_sb = wp.tile([128, CT], F32, tag="temp")
            nc.gpsimd.dma_start(out=temp_sb, in_=io["temp_c"][:, :])

            magic_i = wp.tile([128, NT], I32, tag="magic")
            nc.vector.memset(magic_i, RSQRT_MAGIC)
            c1p5 = wp.tile([128, NT], F32, tag="c1p5")
            nc.vector.memset(c1p5, 1.5)

            def wload(name, kt_tiles, cols, tag=None, dt=F16):
                tile = wp.tile([128, kt_tiles, cols], dt, tag=tag or name)
                src = io[name][:, :].rearrange("(a p) o -> p a o", p=128)
                nc.gpsimd.dma_start(out=tile, in_=src)
                return tile

            wkq_sb = wload("wkq_t", CT, 2 * C)
            wr_sb = wload("wr_t", CT, C)
            wv_sb = wload("wv_t", CT, C)     # (wv*g1).T, [c, d]
            late_w = {}

            def load_late_1():
                late_w["w1"] = wload("w1_t", CT, DFF)
                late_w["w2"] = wload("w2_t", FT, C, dt=F8)
                late_w["qkw"] = wload("qk_t", CT, 2 * C)
                late_w["vw"] = wload("v_t", CT, C)
                late_w["pw"] = wload("p_t", CT, C)

            def load_late_2():
                late_w["w3"] = wload("w3_t", CT, DFF)
                late_w["w4"] = wload("w4_t", FT, C, dt=F8)

            # ---------------- helpers ----------------
            def rsqrt_dve(out, in_ap, n, scratch_tag):
                """out[128, n] f32 = 1/sqrt(in_ap) via bit-trick + 1 NR step."""
                t0 = tp.tile([128, n], F32, tag=scratch_tag, name=f"{scratch_tag}_t0")
                nc.vector.tensor_scalar_add(t0, in_ap, EPS_LN)
                sh = tp.tile([128, n], I32, tag=scratch_tag + "i",
                             name=f"{scratch_tag}_sh")
                nc.vector.tensor_scalar(out=sh, in0=t0[:, :].bitcast(I32),
                                        scalar1=1, scalar2=None,
                                        op0=OP.logical_shift_right)
                y0i = tp.tile([128, n], I32, tag=scratch_tag + "i2",
                              name=f"{scratch_tag}_y0i")
                nc.vector.scalar_tensor_tensor(
                    out=y0i, in0=sh, scalar=-1, in1=magic_i[:, 0:n],
                    op0=OP.mult, op1=OP.add)
                y0 = y0i[:, :].bitcast(F32)
                # NR: y1 = y0 * (1.5 - 0.5*t0*y0^2)
                a = tp.tile([128, n], F32, tag=scratch_tag + "a",
                            name=f"{scratch_tag}_a")
                nc.vector.tensor_mul(a, y0, y0)
                nc.vector.tensor_mul(a, a, t0)          # t0*y0^2
                nc.vector.scalar_tensor_tensor(
                    out=a, in0=a, scalar=-0.5, in1=c1p5[:, 0:n],
                    op0=OP.mult, op1=OP.add)            # 1.5 - 0.5*t0*y0^2
                nc.vector.tensor_mul(out, y0, a)

            def layer_norm_cm(src, s, tag):
                """LN of token-major src [128, NT, C] f16 -> channel-major
                [128, CT, TQ] f16 via DMA-transpose."""
                # block layout: out[c_lo, t_tile, ct, t_lo]; process halves
                out = ap.tile([128, NT, CT, 128], F16, tag=f"lncm_{s}", bufs=2,
                              name=f"lncm_{tag}")
                slab = tp.tile([128, NT, C], F16, tag="ln_slab", bufs=2,
                               name=f"slab_{tag}")
                st6 = tp.tile([128, NT, 6], F16, tag="ln_st6", name=f"st6_{tag}")
                rsig = tp.tile([128, NT], F32, tag="ln_rsig", name=f"rsig_{tag}")
                nm = tp.tile([128, NT], F32, tag="ln_nm", name=f"nm_{tag}")
                hh = NT // 2
                for half in range(2):
                    t0 = half * hh
                    for t in range(t0, t0 + hh):
                        nc.vector.bn_stats(out=st6[:, t, :], in_=src[:, t, :])
                    sl = slice(t0, t0 + hh)
                    m = tp.tile([128, NT], F32, tag="ln_m", name=f"m_{tag}",
                                bufs=2)
                    dm = tp.tile([128, NT], F32, tag="ln_dm", name=f"dm_{tag}",
                                 bufs=2)
                    v = tp.tile([128, NT], F32, tag="ln_v", name=f"v_{tag}",
                                bufs=2)
                    # mean = (m_e + m_o)/2 ; var = (c*v_e + c*v_o)/C + dm^2
                    nc.vector.scalar_tensor_tensor(
                        out=m[:, sl], in0=st6[:, sl, 1], scalar=1.0,
                        in1=st6[:, sl, 4], op0=OP.bypass, op1=OP.add)
                    nc.vector.tensor_scalar_mul(m[:, sl], m[:, sl], 0.5)
                    nc.vector.scalar_tensor_tensor(
                        out=dm[:, sl], in0=st6[:, sl, 1], scalar=1.0,
                        in1=st6[:, sl, 4], op0=OP.bypass, op1=OP.subtract)
                    nc.vector.scalar_tensor_tensor(
                        out=v[:, sl], in0=st6[:, sl, 2], scalar=1.0,
                        in1=st6[:, sl, 5], op0=OP.bypass, op1=OP.add)
                    nc.vector.tensor_mul(dm[:, sl], dm[:, sl], dm[:, sl])
                    # v = v/C + dm/4  (dm holds (m_e-m_o)^2)
                    nc.vector.tensor_scalar(
                        out=dm[:, sl], in0=dm[:, sl], scalar1=0.25,
                        scalar2=None, op0=OP.mult)
                    nc.vector.scalar_tensor_tensor(
                        out=v[:, sl], in0=v[:, sl], scalar=1.0 / C,
                        in1=dm[:, sl], op0=OP.mult, op1=OP.add)
                    rsqrt_dve(rsig[:, sl], v[:, sl], hh, f"rs_{tag}{half}")
                    nc.vector.scalar_tensor_tensor(
                        out=nm[:, sl], in0=m[:, sl],
                        scalar=-1.0, in1=rsig[:, sl],
                        op0=OP.mult, op1=OP.mult)
                    for t in range(t0, t0 + hh):
                        nc.vector.tensor_scalar(
                            out=slab[:, t, :], in0=src[:, t, :],
                            scalar1=rsig[:, t:t + 1], scalar2=nm[:, t:t + 1],
                            op0=OP.mult, op1=OP.add)
                    qn = hh // 2
                    for qq in range(2):
                        q0 = t0 + qq * qn
                        nc.scalar.dma_start_transpose(
                            out=out[:, q0:q0 + qn, :, :].rearrange(
                                "p t c f -> p (t c) f"),
                            in_=slab[:, q0:q0 + qn, :].rearrange(
                                "p t c -> p (t c)"))
                return out

            # ================= per-stream stages =================
            def ea_pre(s, n1cm):
                """K/Q proj, exps, k-softmax scale, S partial accum, CC issue."""
                ps_s01 = pacc.tile([128, 2 * C], F32, tag="ps_s01",
                                   name=f"ps_s01_{s}")
                ps_s0 = ps_s01[:, 0:C]
                ps_s1 = ps_s01[:, C:2 * C]
                kq = ap.tile([128, NT, 2 * C], F16, tag=f"kq_{s}", name=f"kq_{s}")
                ksums = tp.tile([128, NT], F32, tag="ksums", name=f"ksums_{s}")
                rinv = tp.tile([128, NT], F32, tag="rinv", name=f"rinv_{s}")
                hh = NT // 2
                for half in range(2):
                    t0 = half * hh
                    for t in range(t0, t0 + hh):
                        ps = pmm.tile([128, 2 * C], F32, tag="mm")
                        for kt in range(CT):
                            nc.tensor.matmul(ps, n1cm[:, t, kt, :],
                                             wkq_sb[:, kt, :], start=(kt == 0),
                                             stop=(kt == CT - 1))
                        nc.scalar.activation(kq[:, t, :], ps, AF.Exp)
                    sl = slice(t0, t0 + hh)
                    nc.vector.tensor_reduce(ksums[:, sl], kq[:, sl, 0:C],
                                            axis=AX.X, op=OP.add)
                    nc.vector.reciprocal(rinv[:, sl], ksums[:, sl])
                    for t in range(t0, t0 + hh):
                        nc.vector.tensor_scalar_mul(kq[:, t, 0:C], kq[:, t, 0:C],
                                                    rinv[:, t:t + 1])
                    for t in range(t0, t0 + hh):
                        st, sp = (t == 0), (t == NT - 1)
                        nc.tensor.matmul(ps_s0, kq[:, t, C:C + 128],
                                         kq[:, t, 0:C], start=st, stop=sp)
                        nc.tensor.matmul(ps_s1, kq[:, t, C + 128:2 * C],
                                         kq[:, t, 0:C], start=st, stop=sp)
                ea_tx = stg.tile([128, 2 * C], F8, tag=f"ea_tx_{s}")
                with nc.allow_low_precision(reason="fp8 collective payload"):
                    nc.vector.tensor_copy(ea_tx[:, 0:C], ps_s0)
                    nc.vector.tensor_copy(ea_tx[:, C:2 * C], ps_s1)
                nc.sync.dma_start(
                    out=cc[f"ea_in_{s}"][:].rearrange("(p f) -> p f", p=128),
                    in_=ea_tx)
                nc.gpsimd.collective_compute(
                    "AllGather", OP.bypass, replica_groups=REPLICA_GROUPS,
                    ins=[cc[f"ea_in_{s}"][:]], outs=[cc[f"ea_out_{s}"][:]])
                # V channel-major — independent of the collective, fills the gap
                Vcm = ap.tile([128, CT, TQ], F16, tag=f"Vcm_{s}", name=f"Vcm_{s}")
                for dt_ in range(CT):
                    for ch in range(NCH):
                        ps = pmm.tile([128, 512], F32, tag="mm")
                        for kt in range(CT):
                            nc.tensor.matmul(
                                ps, wv_sb[:, kt, dt_ * 128:(dt_ + 1) * 128],
                                n1cm[:, 4 * ch:4 * ch + 4, kt, :],
                                start=(kt == 0), stop=(kt == CT - 1))
                        nc.scalar.activation(
                            Vcm[:, dt_, ch * 512:(ch + 1) * 512], ps, AF.Identity)
                return Vcm

            def ea_post(s, Vcm, x_res):
                """Sum gathered S, fold colsum+wr+wv, att, residual add1."""
                g = stg.tile([128, NRANKS, 2 * C], F8, tag="ea_rx", bufs=2,
                             name=f"ea_rx_{s}")
                nc.sync.dma_start(
                    out=g, in_=cc[f"ea_out_{s}"][:].rearrange(
                        "(r p f) -> p r f", p=128, r=NRANKS))
                s_tot = stg.tile([128, 2 * C], F16, tag=f"s_tot_{s}")
                if NRANKS == 2:
                    nc.vector.tensor_add(s_tot, g[:, 0, :], g[:, 1, :])
                else:
                    st01 = tp.tile([128, 2 * C], F16, tag="st01")
                    st23 = tp.tile([128, 2 * C], F16, tag="st23")
                    nc.vector.tensor_add(st01, g[:, 0, :], g[:, 1, :])
                    nc.vector.tensor_add(st23, g[:, 2, :], g[:, 3, :])
                    nc.vector.tensor_add(s_tot, st01, st23)
                # q-denominators: row-sums of each e-half block
                qden = tp.tile([128, CT], F32, tag="qden")
                nc.vector.tensor_reduce(
                    qden, s_tot[:, :].rearrange("p (e o) -> p e o", e=CT),
                    axis=AX.X, op=OP.add)
                cinv = tp.tile([128, CT], F32, tag="cinv")
                nc.vector.reciprocal(cinv, qden)
                wrs = stg.tile([128, CT, C], F16, tag=f"wrs_{s}")
                for et in range(CT):
                    nc.vector.tensor_scalar_mul(wrs[:, et, :], wr_sb[:, et, :],
                                                cinv[:, et:et + 1])
                # S2[d, o] = sum_e S[e, d] * wrs[e, o]
                s2_sb = stg.tile([128, CT, C], F16, tag=f"s2_{s}")
                for mt in range(CT):
                    ps = pmm.tile([128, C], F32, tag="mm")
                    for et in range(CT):
                        nc.tensor.matmul(
                            ps, s_tot[:, et * C + mt * 128: et * C + (mt + 1) * 128],
                            wrs[:, et, :], start=(et == 0), stop=(et == CT - 1))
                    nc.scalar.activation(s2_sb[:, mt, :], ps, AF.Identity)
                # att = V @ S2 ; add1 = x + att
                add1 = ap.tile([128, NT, C], F16, tag=f"resid_{s}", bufs=2,
                               name=f"add1_{s}")
                for t in range(NT):
                    ps = pmm.tile([128, C], F32, tag="mm")
                    for dt_ in range(CT):
                        nc.tensor.matmul(ps, Vcm[:, dt_, t * 128:(t + 1) * 128],
                                         s2_sb[:, dt_, :], start=(dt_ == 0),
                                         stop=False)
                    nc.tensor.matmul(ps, ident, x_res[:, t, :], start=False,
                                     stop=True, skip_group_check=True)
                    nc.scalar.activation(add1[:, t, :], ps, AF.Identity)
                return add1

            def mlp(s, src_cm, resid, w_a, w_b, out_dram):
                """resid + W_b.T @ gelu(W_a.T @ src_cm); fc2 in fp8 DoubleRow.
                If out_dram, stream f32 result to DRAM, else return f16 tile."""
                h = ap.tile([128, FT, TQ], F8, tag=f"hbuf_{s}")
                for ft in range(FT):
                    for ch in range(NCH):
                        ps = pmm.tile([128, 512], F32, tag="mm")
                        for kt in range(CT):
                            nc.tensor.matmul(
                                ps, w_a[:, kt, ft * 128:(ft + 1) * 128],
                                src_cm[:, 4 * ch:4 * ch + 4, kt, :],
                                start=(kt == 0), stop=(kt == CT - 1))
                        nc.scalar.activation(
                            h[:, ft, ch * 512:(ch + 1) * 512], ps, AF.Gelu)
                out = None
                if out_dram is None:
                    out = ap.tile([128, NT, C], F16, tag=f"resid_{s}", bufs=2,
                                  name=f"add2_{s}")
                ysb = None
                if out_dram is not None:
                    ysb = tp.tile([128, 4, C], F32, tag="ysb", bufs=2,
                                  name=f"ysb_{s}")
                for t in range(NT):
                    ps = pmm.tile([128, C], F32, tag="mm")
                    for fp in range(FT // 2):
                        nc.tensor.matmul(
                            ps, h[:, 2 * fp:2 * fp + 2, t * 128:(t + 1) * 128],
                            w_b[:, 2 * fp:2 * fp + 2, :],
                            start=(fp == 0), stop=False,
                            perf_mode=mybir.MatmulPerfMode.DoubleRow)
                    nc.tensor.matmul(ps, ident, resid[:, t, :], start=False,
                                     stop=True, skip_group_check=True)
                    if out_dram is not None:
                        if t % 2 == 0:
                            nc.scalar.activation(ysb[:, t % 4, :], ps, AF.Identity)
                        else:
                            nc.vector.tensor_copy(ysb[:, t % 4, :], ps)
                        if t % 4 == 3:
                            nc.sync.dma_start(
                                out=out_dram[:, :].rearrange(
                                    "(tt p) c -> p tt c", p=128)[:, t - 3:t + 1, :],
                                in_=ysb)
                            if t < NT - 1:
                                ysb = tp.tile([128, 4, C], F32, tag="ysb",
                                              bufs=2, name=f"ysb_{s}{t}")
                    else:
                        if t % 2 == 0:
                            nc.scalar.activation(out[:, t, :], ps, AF.Identity)
                        else:
                            nc.vector.tensor_copy(out[:, t, :], ps)
                return out

            def ca_pre(s, n3cm):
                """qk proj + norms + gram partials + v_cm; CC issue."""
                ps_a01 = pacc.tile([128, 2 * C], F32, tag="ps_a01",
                                   name=f"ps_a01_{s}")
                ps_a0 = ps_a01[:, 0:C]
                ps_a1 = ps_a01[:, C:2 * C]
                ps_nrm = pacc.tile([128, 2 * C], F32, tag="ps_nrm", name=f"ps_nrm_{s}")
                for t in range(NT):
                    st, sp = (t == 0), (t == NT - 1)
                    ps = pmm.tile([128, 2 * C], F32, tag="mm")
                    for kt in range(CT):
                        nc.tensor.matmul(ps, n3cm[:, t, kt, :],
                                         late_w["qkw"][:, kt, :], start=(kt == 0),
                                         stop=(kt == CT - 1))
                    qkt = tp.tile([128, 2 * C], F16, tag="qkt", bufs=4)
                    if t % 2 == 0:
                        nc.scalar.activation(qkt, ps, AF.Identity)
                    else:
                        nc.vector.tensor_copy(qkt, ps)
                    sq = tp.tile([128, 2 * C], F16, tag="sq", bufs=4)
                    nc.vector.tensor_mul(sq, qkt, qkt)
                    nc.tensor.matmul(ps_nrm[0:1, :], ones_col, sq, start=st, stop=sp)
                    nc.tensor.matmul(ps_a0, qkt[:, 0:128], qkt[:, C:2 * C],
                                     start=st, stop=sp)
                    nc.tensor.matmul(ps_a1, qkt[:, 128:256], qkt[:, C:2 * C],
                                     start=st, stop=sp)
                # pack: per-head diag 32x32 gram blocks + q/k sumsq columns
                W = 2 * HD + 2 * CT
                ca_tx = stg.tile([128, W], F16, tag=f"ca_tx_{s}")
                for hh in range(H_CH):
                    ct, r0 = hh // 4, (hh % 4) * HD
                    src_ps = ps_a0 if ct == 0 else ps_a1
                    nc.vector.tensor_copy(ca_tx[r0:r0 + HD, ct * HD:(ct + 1) * HD],
                                          src_ps[r0:r0 + HD, hh * HD:(hh + 1) * HD])
                nrm_sb = tp.tile([1, 2 * C], F32, tag="nrm_sb")
                nc.vector.tensor_copy(nrm_sb, ps_nrm[0:1, :])
                ps_fl = pmm.tile([128, 2 * CT], F32, tag="mm")
                for i in range(2 * CT):
                    nc.tensor.transpose(ps_fl[:, i:i + 1],
                                        nrm_sb[0:1, i * 128:(i + 1) * 128],
                                        ident32[0:1, 0:1])
                nc.vector.tensor_copy(ca_tx[:, 2 * HD:W], ps_fl)
                nc.sync.dma_start(
                    out=cc[f"ca_in_{s}"][:].rearrange("(p f) -> p f", p=128),
                    in_=ca_tx)
                # v channel-major — independent of the collective
                vcm = ap.tile([128, CT, TQ], F16, tag=f"vcm_{s}")
                for et in range(CT):
                    for ch in range(NCH):
                        ps = pmm.tile([128, 512], F32, tag="mm")
                        for kt in range(CT):
                            nc.tensor.matmul(
                                ps, late_w["vw"][:, kt, et * 128:(et + 1) * 128],
                                n3cm[:, 4 * ch:4 * ch + 4, kt, :],
                                start=(kt == 0), stop=(kt == CT - 1))
                        nc.scalar.activation(vcm[:, et, ch * 512:(ch + 1) * 512],
                                             ps, AF.Identity)
                nc.gpsimd.collective_compute(
                    "AllGather", OP.bypass, replica_groups=REPLICA_GROUPS,
                    ins=[cc[f"ca_in_{s}"][:]], outs=[cc[f"ca_out_{s}"][:]])
                return vcm

            def ca_post(s, vcm, resid):
                """Gathered gram -> per-head softmax -> fold with proj -> out."""
                W = 2 * HD + 2 * CT
                g = stg.tile([128, NRANKS, W], F16, tag="ca_rx", bufs=2,
                             name=f"ca_rx_{s}")
                nc.sync.dma_start(
                    out=g, in_=cc[f"ca_out_{s}"][:].rearrange(
                        "(r p f) -> p r f", p=128, r=NRANKS))
                tot = stg.tile([128, W], F32, tag=f"ca_tot_{s}")
                if NRANKS == 2:
                    nc.vector.tensor_add(tot, g[:, 0, :], g[:, 1, :])
                else:
                    t01 = tp.tile([128, W], F16, tag="ca01")
                    t23 = tp.tile([128, W], F16, tag="ca23")
                    nc.vector.tensor_add(t01, g[:, 0, :], g[:, 1, :])
                    nc.vector.tensor_add(t23, g[:, 2, :], g[:, 3, :])
                    nc.vector.tensor_add(tot, t01, t23)
                # inverse norms (rsqrt of summed squares), cols: q ct0,ct1,k ct0,ct1
                invn = tp.tile([128, 2 * CT], F32, tag="invn", name=f"invn_{s}")
                rsqrt_dve(invn, tot[:, 2 * HD:W], 2 * CT, f"can_{s}")
                invq = tp.tile([128, CT], F32, tag="invq", name=f"invq_{s}")
                nc.vector.tensor_mul(invq, invn[:, 0:CT], temp_sb)
                # k-inv-norm row broadcast into [128, C] via PE
                ps_kf = pmm.tile([128, C], F32, tag="mm", name=f"pskf_{s}")
                for ct in range(CT):
                    nc.tensor.transpose(ps_kf[0:1, ct * 128:(ct + 1) * 128],
                                        invn[:, CT + ct:CT + ct + 1], ident32)
                ikr = tp.tile([1, C], F16, tag="ikr", name=f"ikr_{s}")
                nc.vector.tensor_copy(ikr, ps_kf[0:1, :])
                ps_bk = pmm.tile([128, C], F32, tag="mm", name=f"psbk_{s}")
                nc.tensor.matmul(ps_bk, ones_row, ikr, start=True, stop=True)
                bk_sb = tp.tile([128, C], F32, tag="bk", name=f"bk_{s}")
                nc.vector.tensor_copy(bk_sb, ps_bk)

                attn_l = tp.tile([128, 2 * HD], F32, tag="attn_l", name=f"al_{s}")
                for hh in range(H_CH):
                    ct, r0 = hh // 4, (hh % 4) * HD
                    nc.vector.scalar_tensor_tensor(
                        out=attn_l[r0:r0 + HD, ct * HD:(ct + 1) * HD],
                        in0=tot[r0:r0 + HD, ct * HD:(ct + 1) * HD],
                        scalar=invq[r0:r0 + HD, ct:ct + 1],
                        in1=bk_sb[r0:r0 + HD, hh * HD:(hh + 1) * HD],
                        op0=OP.mult, op1=OP.mult)
                # batched per-head softmax on the compact [128, CT, HD] layout
                attn_c = stg.tile([128, CT, HD], F16, tag=f"attn_c_{s}")
                mx = tp.tile([128, CT], F32, tag="camx", name=f"mx_{s}")
                sm = tp.tile([128, CT], F32, tag="casm", name=f"sm_{s}")
                rv = tp.tile([128, CT], F32, tag="carv", name=f"rv_{s}")
                nc.vector.tensor_reduce(
                    mx, attn_l[:, :].rearrange("p (c h) -> p c h", c=CT),
                    axis=AX.X, op=OP.max, negate=True)
                for ct in range(CT):
                    nc.scalar.activation(attn_c[:, ct, :],
                                         attn_l[:, ct * HD:(ct + 1) * HD],
                                         AF.Exp, bias=mx[:, ct:ct + 1], scale=1.0,
                                         accum_out=sm[:, ct:ct + 1])
                nc.vector.reciprocal(rv, sm)
                for ct in range(CT):
                    nc.vector.tensor_scalar_mul(attn_c[:, ct, :], attn_c[:, ct, :],
                                                rv[:, ct:ct + 1])
                # scatter to block-diagonal slabs; M2[d,o] = sum_e A[e,d] P[e,o]
                attn_e = stg.tile([128, CT, 128], F16, tag=f"attn_e_{s}")
                nc.vector.memset(attn_e, 0.0)
                for hh in range(H_CH):
                    ct, r0 = hh // 4, (hh % 4) * HD
                    nc.vector.tensor_copy(attn_e[r0:r0 + HD, ct, r0:r0 + HD],
                                          attn_c[r0:r0 + HD, ct, :])
                m2_sb = stg.tile([128, CT, C], F16, tag=f"m2_{s}")
                for ct in range(CT):
                    ps = pmm.tile([128, C], F32, tag="mm")
                    nc.tensor.matmul(ps, attn_e[:, ct, :], late_w["pw"][:, ct, :],
                                     start=True, stop=True)
                    nc.scalar.activation(m2_sb[:, ct, :], ps, AF.Identity)
                # out[t, o] = sum_d vcm[d, t] M2[d, o] ; add3 = resid + out
                add3 = ap.tile([128, NT, C], F16, tag=f"resid_{s}", bufs=2,
                               name=f"add3_{s}")
                for t in range(NT):
                    ps = pmm.tile([128, C], F32, tag="mm")
                    for dt in range(CT):
                        nc.tensor.matmul(ps, vcm[:, dt, t * 128:(t + 1) * 128],
                                         m2_sb[:, dt, :], start=(dt == 0),
                                         stop=False)
                    nc.tensor.matmul(ps, ident, resid[:, t, :], start=False,
                                     stop=True, skip_group_check=True)
                    nc.scalar.activation(add3[:, t, :], ps, AF.Identity)
                return add3

            # ================= interleaved schedule =================
            n1, Vcm = {}, {}
            for s in STREAMS:
                n1[s] = layer_norm_cm(x_sb[s], s, f"n1{s}")
                Vcm[s] = ea_pre(s, n1[s])
            load_late_1()
            load_late_2()
            add1, add2, n3, vcm = {}, {}, {}, {}
            for s in STREAMS:
                add1[s] = ea_post(s, Vcm[s], x_sb[s])
                n2 = layer_norm_cm(add1[s], s, f"n2{s}")
                add2[s] = mlp(s, n2, add1[s], late_w["w1"], late_w["w2"], None)
                n3[s] = layer_norm_cm(add2[s], s, f"n3{s}")
                vcm[s] = ca_pre(s, n3[s])
            for s in STREAMS:
                add3 = ca_post(s, vcm[s], add2[s])
                n4 = layer_norm_cm(add3, s, f"n4{s}")
                mlp(s, n4, add3, late_w["w3"], late_w["w4"], io[f"y_{s}"])

    nc.compile()
    _CACHE["nc"] = nc
    return nc


def prep_host(inputs):
    """Fold LN gammas into weights; fp16 staged host arrays (shared)."""
    f = lambda k: np.asarray(inputs[k], np.float32)
    for k in ("ln1_b", "ln2_b", "ln3_b", "ln4_b", "m1_b2", "m2_b2", "proj_b",
              "m1_b1", "m2_b1"):
        assert np.abs(f(k)).max() == 0.0, f"{k} nonzero; bias path not emitted"
    g1, g2, g3, g4 = f("ln1_g"), f("ln2_g"), f("ln3_g"), f("ln4_g")
    qkv_w = f("qkv_w")
    h = lambda a: np.ascontiguousarray(a).astype(np.float16)
    try:
        import ml_dtypes
        _f8 = ml_dtypes.float8_e4m3
    except ImportError:
        _f8 = None
    f8 = lambda a: np.ascontiguousarray(a).astype(_f8)
    return {
        "wkq_t": h(np.concatenate(
            [(f("wk") * g1[None, :]).T, (f("wq") * g1[None, :]).T], axis=1)),
        "wr_t": h(f("wr").T),
        "wv_t": h((f("wv") * g1[None, :]).T),
        "qk_t": h((qkv_w[: 2 * C] * g3[None, :]).T),
        "v_t": h((qkv_w[2 * C:] * g3[None, :]).T),
        "p_t": h(f("proj_w").T),
        "w1_t": h((f("m1_w1") * g2[None, :]).T),
        "w2_t": f8(f("m1_w2").T),
        "w3_t": h((f("m2_w1") * g4[None, :]).T),
        "w4_t": f8(f("m2_w2").T),
        "temp_c": np.ascontiguousarray(
            np.repeat(f("temperature").reshape(H_CH), HD).reshape(CT, 128).T
        ).astype(np.float32),
        "ident": h(np.eye(128)),
        "ones_pc": h(np.ones((128, 1))),
        "ones_pr": h(np.ones((1, 128))),
    }


def make_in_maps(inputs):
    shared = prep_host(inputs)
    x = np.asarray(inputs["x"], np.float32)
    in_maps = []
    for c in range(NCORES):
        m = dict(shared)
        if DUAL:
            g, q = c // 4, c % 4
            def perm(a):
                a = a.reshape(NT, 128, C).transpose(1, 0, 2)
                return np.ascontiguousarray(
                    a.reshape(128 * NT, C)).astype(np.float16)
            m["x_a"] = perm(x[2 * g, q * TQ:(q + 1) * TQ, :])
            m["x_b"] = perm(x[2 * g + 1, q * TQ:(q + 1) * TQ, :])
        else:
            b, hf = c // 2, c % 2
            xa = x[b, hf * TQ:(hf + 1) * TQ, :].reshape(NT, 128, C)
            m["x_a"] = np.ascontiguousarray(
                xa.transpose(1, 0, 2).reshape(128 * NT, C)).astype(np.float16)
        in_maps.append(m)
    return in_maps


def assemble(results):
    y = np.empty((B, N, C), np.float32)
    for c in range(NCORES):
        if DUAL:
            g, q = c // 4, c % 4
            y[2 * g, q * TQ:(q + 1) * TQ, :] = results[c]["y_a"]
            y[2 * g + 1, q * TQ:(q + 1) * TQ, :] = results[c]["y_b"]
        else:
            b, hf = c // 2, c % 2
            y[b, hf * TQ:(hf + 1) * TQ, :] = results[c]["y_a"]
    return y


def kernel(**inputs):
    from concourse.bass_utils import run_bass_kernel_spmd

    nc = build_program()
    in_maps = make_in_maps(inputs)
    res = run_bass_kernel_spmd(nc, in_maps, list(range(NCORES)))
    return assemble(res.results)
